# revision 4
# baseline (speedup 1.0000x reference)
"""Trainium2 Bass kernel for nn_DiscriminatorBlock_38878043963811.

Strategy
--------
Data-parallel over batch: 16 images -> 8 cores x 2 images. No collectives.

v2 redesign (vs baseline):
  * Input DMA: host pre-builds ALL 27 (dh, dw, rgb) shifted tap replicas for
    each of the 4 row-chunks -> [108, 32*130] bf16 per image, loaded in ONE
    DMA (cost model charges bytes-per-partition; 9 small DMAs paid 9x).
  * Whole linear path = one K=27 matmul z_pre = M27 @ s27 per 512-px block
    (fromrgb + depthwise v/h + low-rank residual + point 1x1), with the
    lrelu gain and both FIR norms (sqrt(2)/64) folded into M27.
  * prelu evac PSUM->SBUF split across ACT (Prelu) and Pool/DVE
    (stt max(0.2x, x)), writing w-deinterleaved bf16 rows [even64|odd64].
  * FIR order swapped: FIR-h FIRST (DVE stt/add over unit-stride halves,
    3 ops/elem), then FIR-v on PE over the half-width q rows as
    scaled-identity accumulating matmuls (2 streams/row instead of 4 on
    full-width) -> PSUM f32, evac to bf16, DMA out (host upcasts to f32).
"""

import sys

sys.path.insert(0, "/opt/trn_rl_repo")

import numpy as np
import ml_dtypes
from numpy.lib.stride_tricks import as_strided

import concourse.bass as bass
import concourse.bacc as bacc
import concourse.tile as tile
from concourse import mybir
from concourse.bass_utils import run_bass_kernel_spmd

f32 = mybir.dt.float32
bf16 = mybir.dt.bfloat16
AF = mybir.ActivationFunctionType
ALU = mybir.AluOpType

# ---- problem constants (hardcoded; kernel.py must be self-contained) ----
B, IMG_C, IN_C, OUT_C, S = 16, 3, 256, 512, 128
HIDDEN = IN_C
KGEN_IN = 32
KSIZE = 3
N_CORES = 8
B_LOC = B // N_CORES            # 2 images per core
HC = 32                         # z-rows per chunk
NCHUNK = S // HC                # 4 chunks per image
GDW = np.float32(1.0 / np.sqrt(KSIZE))
ACT_SCALE = float(np.sqrt(2.0) / 64.0)
SW = HC * 130                   # 4160 elems per replica partition per chunk

_CACHE = {}


def _sample_weight_np(grid, coeff, gauss_sigma, gauss_x, low_filter):
    """numpy port of reference._sample_weight (fp32)."""
    basis = np.sin(grid * np.float32(2.0 * np.pi)) * np.float32(np.exp(-0.5))
    w = coeff @ basis / np.float32(np.sqrt(HIDDEN))
    w = w - w.mean(dtype=np.float32)
    w = w * (1.0 / np.sqrt(np.mean(w * w, axis=0, keepdims=True, dtype=np.float32) + 1e-8))
    gs = 1.0 + gauss_sigma ** 2 / 5.0
    w = (w * np.exp(-(gauss_x ** 2) / (2.0 * gs))).astype(np.float32)
    nt = low_filter.shape[0]
    T = w.shape[1] - nt + 1
    out = np.empty((w.shape[0], T), np.float32)
    for t in range(T):
        out[:, t] = (w[:, t : t + nt] * low_filter[None, :]).sum(axis=1)
    return out[:, ::2]


def _build_program():
    nc = bacc.Bacc(None, target_bir_lowering=False)
    s_d = nc.declare_dram_parameter("s", [B_LOC * 128, SW], bf16, isOutput=False)
    m27_d = nc.declare_dram_parameter("m27", [128, OUT_C], bf16, isOutput=False)
    id_d = nc.declare_dram_parameter("ident", [128, 256], bf16, isOutput=False)
    out_d = nc.declare_dram_parameter("out", [B_LOC, OUT_C, S // 2, S // 2], bf16,
                                      isOutput=True)

    with tile.TileContext(nc) as tc:
        with (
            tc.tile_pool(name="const", bufs=1) as cpool,
            tc.tile_pool(name="spool", bufs=2) as spool,
            tc.tile_pool(name="ztpool", bufs=3) as ztpool,
            tc.tile_pool(name="t1pool", bufs=3) as t1pool,
            tc.tile_pool(name="qpool", bufs=13) as qpool,
            tc.tile_pool(name="opool", bufs=4) as opool,
            tc.tile_pool(name="scr", bufs=2) as scrpool,
            tc.tile_pool(name="zpsum", bufs=2, space="PSUM") as zpsum,
            tc.tile_pool(name="ovps", bufs=2, space="PSUM") as ovps,
        ):
            # ---- constants ----
            m27t = cpool.tile([128, OUT_C], bf16)
            nc.sync.dma_start(m27t[:], m27_d[:])
            idt = cpool.tile([128, 256], bf16)
            nc.sync.dma_start(idt[:], id_d[:])
            zero64 = cpool.tile([128, 64], bf16)
            nc.vector.memset(zero64[:], 0.0)
            # PE p-state warmup: keep PE busy from t~1us so it reaches full
            # clock before the first real z matmuls
            wp = zpsum.tile([128, 64], f32, tag="zp", name="warm")
            for _ in range(40):
                nc.tensor.matmul(wp[:], idt[:, 0:128], zero64[:],
                                 start=True, stop=True)

            def fir_v9(b, bk, mt, qc, qp):
                """First 9 FIR-v matmuls for (chunk bk, mt); k3B deferred."""
                I1 = idt[:, 0:128]
                I3 = idt[:, 128:256]
                q3 = qc[mt][:].rearrange("p (a two w) -> p a two w", two=2, w=64)
                ov = ovps.tile([128, 1024], f32, tag="ov", name=f"ov{b}_{bk}_{mt}")
                mm = nc.tensor.matmul
                mm(ov[:, 0:512], I3, q3[:, 0:8, 0, :], start=True, stop=False)
                mm(ov[:, 0:512], I3, q3[:, 0:8, 1, :], start=False, stop=False)
                mm(ov[:, 0:512], I1, q3[:, 1:9, 0, :], start=False, stop=False)
                mm(ov[:, 64:512], I1, q3[:, 0:7, 1, :], start=False, stop=True)
                prev_src = (qp[mt][:].rearrange("p (a two w) -> p a two w",
                                                two=2, w=64)[:, 15:16, 1, :]
                            if qp is not None else zero64[:, 0:64])
                mm(ov[:, 0:64], I1, prev_src, start=False, stop=True)
                mm(ov[:, 512:1024], I1, q3[:, 7:15, 1, :], start=True, stop=False)
                mm(ov[:, 512:1024], I3, q3[:, 8:16, 0, :], start=False, stop=False)
                mm(ov[:, 512:1024], I3, q3[:, 8:16, 1, :], start=False, stop=False)
                mm(ov[:, 512:960], I1, q3[:, 9:16, 0, :], start=False, stop=True)
                return ov

            def fir_v_tail(b, bk, mt, ov, qn, on_act=False):
                """Deferred k3B tap + evac + store for (chunk bk, mt)."""
                I1 = idt[:, 0:128]
                next_src = (qn[mt][:].rearrange("p (a two w) -> p a two w",
                                                two=2, w=64)[:, 0:1, 0, :]
                            if qn is not None else zero64[:, 0:64])
                nc.tensor.matmul(ov[:, 960:1024], I1, next_src,
                                 start=False, stop=True)
                o2 = opool.tile([128, 1024], bf16, tag="o2",
                                name=f"o2_{b}_{bk}_{mt}")
                if on_act:
                    nc.scalar.activation(o2[:], ov[:], AF.Copy, bias=0.0,
                                         scale=1.0)
                else:
                    nc.vector.tensor_copy(o2[:], ov[:])
                nc.sync.dma_start(
                    out_d[b, mt * 128 : (mt + 1) * 128, bk * 16 : bk * 16 + 16, :],
                    o2[:].rearrange("p (r w) -> p r w", w=64),
                )

            stiles = {}
            for b in range(B_LOC):
                s_a = spool.tile([96, SW], bf16, tag="sa", name=f"sa{b}")
                nc.sync.dma_start(s_a[:, 0:1040], s_d[b * 128 : b * 128 + 96, 0:1040])
                nc.sync.dma_start(s_a[:, 1040:SW], s_d[b * 128 : b * 128 + 96, 1040:SW])
                s_b = spool.tile([32, SW], bf16, tag="sb", name=f"sb{b}")
                nc.sync.dma_start(s_b[:], s_d[b * 128 + 96 : b * 128 + 128])
                stiles[b] = (s_a, s_b)

            qhist = {}      # (b, j) -> q tiles
            fv9_pend = []   # fv9 units awaiting issue: (b, bk, mt)
            k3b_pend = []   # deferred tails: (b, bk, mt, ov, qn_key)

            def flush_k3b(limit, on_act=False):
                while len(k3b_pend) > limit:
                    fb, fbk, fmt, fov, qn_key = k3b_pend[0]
                    qn = qhist.get(qn_key)
                    if qn_key is not None and (qn is None or len(qn) <= fmt):
                        break   # next-chunk q tile not built yet; retry later
                    k3b_pend.pop(0)
                    fir_v_tail(fb, fbk, fmt, fov, qn, on_act)
            for b in range(B_LOC):
                s_a, s_b = stiles[b]
                for j in range(NCHUNK):
                    sbase = 32 * j if j < 3 else 0
                    stile = s_a if j < 3 else s_b
                    s93v = stile[sbase : sbase + 27, :].rearrange(
                        "p (r w) -> p r w", w=130)
                    qts = []
                    qhist[(b, j)] = qts
                    for mt in range(4):
                        zt = ztpool.tile([128, HC * 128], bf16, tag="zt",
                                         name=f"zt{b}_{j}_{mt}")
                        z3 = zt[:].rearrange("p (r w) -> p r w", w=128)
                        zdst4 = zt[:].rearrange("p (r par w2) -> p r par w2",
                                                par=2, w2=64)
                        for t in range(4):
                            zp = zpsum.tile([128, 1024], f32, tag="zp",
                                            name=f"zp{b}_{j}_{mt}_{t}")
                            for nn in range(2):
                                r0 = 8 * t + 4 * nn
                                nc.tensor.matmul(
                                    zp[:, nn * 512 : nn * 512 + 512],
                                    m27t[sbase : sbase + 27,
                                         mt * 128 : mt * 128 + 128],
                                    s93v[:, r0 : r0 + 4, 0:128],
                                    start=True, stop=True)
                            zpv = zp[:].rearrange("p (r w2 two) -> p r two w2",
                                                  two=2, w2=64)
                            rows = zdst4[:, 8 * t : 8 * t + 8, :, :]
                            if False and mt == 2 and t == 1 and (j % 2 == 0):
                                # two-stage prelu on DVE (frees one ACT slot)
                                sc = scrpool.tile([128, 1024], f32, tag="sc",
                                                  name=f"sc{b}_{j}")
                                nc.vector.tensor_copy(sc[:], zp[:])
                                scv = sc[:].rearrange(
                                    "p (r w2 two) -> p r two w2", two=2, w2=64)
                                nc.vector.scalar_tensor_tensor(
                                    rows, scv, 0.2, scv, ALU.mult, ALU.max)
                            else:
                                nc.scalar.activation(rows, zpv, AF.Prelu,
                                                     bias=0.0, scale=1.0, alpha=0.2)
                        t1t = t1pool.tile([128, HC * 64], bf16, tag="t1",
                                          name=f"t1{b}_{j}_{mt}")
                        qt = qpool.tile([128, HC * 64], bf16, tag="q",
                                        name=f"q{b}_{j}_{mt}")
                        t13 = t1t[:].rearrange("p (r w) -> p r w", w=64)
                        q3 = qt[:].rearrange("p (r w) -> p r w", w=64)
                        e = z3[:, :, 0:64]
                        o = z3[:, :, 64:128]
                        # row 0 first (feeds the lag-1 deferred k3B tap)
                        nc.vector.tensor_add(t13[:, 0:1, :], e[:, 0:1, :],
                                             o[:, 0:1, :])
                        nc.vector.scalar_tensor_tensor(
                            q3[:, 0:1, 1:64], t13[:, 0:1, 1:64], 3.0,
                            z3[:, 0:1, 64:127], ALU.mult, ALU.add)
                        nc.vector.tensor_scalar_mul(q3[:, 0:1, 0:1],
                                                    t13[:, 0:1, 0:1], 3.0)
                        nc.vector.tensor_add(q3[:, 0:1, 0:63], q3[:, 0:1, 0:63],
                                             z3[:, 0:1, 1:64])
                        # bulk rows 1..31
                        nc.gpsimd.tensor_add(t13[:, 1:32, :], e[:, 1:32, :],
                                             o[:, 1:32, :])
                        nc.vector.scalar_tensor_tensor(
                            q3[:, 1:32, 1:64], t13[:, 1:32, 1:64], 3.0,
                            z3[:, 1:32, 64:127], ALU.mult, ALU.add)
                        nc.vector.tensor_scalar_mul(q3[:, 1:32, 0:1],
                                                    t13[:, 1:32, 0:1], 3.0)
                        nc.gpsimd.tensor_add(q3[:, 1:32, 0:63],
                                             q3[:, 1:32, 0:63],
                                             z3[:, 1:32, 1:64])
                        qts.append(qt)

                        # issue one pending fv9 unit (lag: prev chunk's units)
                        if fv9_pend:
                            ub, ubk, umt = fv9_pend.pop(0)
                            qc = qhist[(ub, ubk)]
                            qp = qhist.get((ub, ubk - 1))
                            ov = fir_v9(ub, ubk, umt, qc, qp)
                            # qn = q of the chunk AFTER ubk (same image only)
                            qn_key = (ub, ubk + 1) if ubk + 1 < NCHUNK else None
                            k3b_pend.append((ub, ubk, umt, ov, qn_key))
                            flush_k3b(1)
                    fv9_pend.extend((b, j, mt) for mt in range(4))
                    # drop q tiles no longer needed
                    qhist.pop((b, j - 3), None)
            # drain remaining fv9 units and tails (evacs on ACT: it idles here)
            di = 0
            while fv9_pend:
                ub, ubk, umt = fv9_pend.pop(0)
                ov = fir_v9(ub, ubk, umt, qhist[(ub, ubk)],
                            qhist.get((ub, ubk - 1)))
                qn_key = (ub, ubk + 1) if ubk + 1 < NCHUNK else None
                k3b_pend.append((ub, ubk, umt, ov, qn_key))
                flush_k3b(1, on_act=(di % 2 == 0))
                di += 1
            flush_k3b(0, on_act=False)

    nc.compile()
    return nc


def kernel(**inputs):
    inputs = {k: np.asarray(v) for k, v in inputs.items()}
    img = inputs["img"].astype(np.float32)
    assert img.shape == (B, IMG_C, S, S)

    # ---- host-side weight generation (tiny) ----
    freqs = inputs["freqs"].astype(np.float32)
    phases = inputs["phases"].astype(np.float32)
    g = ((np.arange(KGEN_IN, dtype=np.float32) - (KGEN_IN - 1) / 2.0)
         * np.float32(2.0 / (KGEN_IN + 1)))
    gsig = np.float32(inputs["gauss_sigma"])
    gx = inputs["gauss_x"].astype(np.float32)
    lf = inputs["low_filter"].astype(np.float32)
    hz = _sample_weight_np(freqs[:, 0:1] * g[None, :] + phases[:, None],
                           inputs["hz_outdim"].astype(np.float32), gsig, gx, lf)
    vt = _sample_weight_np(freqs[:, 1:2] * g[None, :] + phases[:, None],
                           inputs["vt_outdim"].astype(np.float32), gsig, gx, lf)

    Wfr = inputs["fromrgb_w"][:, :, 0, 0].astype(np.float32) * np.float32(1.0 / np.sqrt(IMG_C))
    assert np.abs(Wfr).sum(1).max() < 250.0, "fromrgb clamp would be active"
    assert np.all(inputs["fromrgb_b"] == 0.0), "nonzero fromrgb bias unsupported"
    assert np.all(inputs["point_b"] == 0.0), "nonzero point bias unsupported"

    # k27[(d*3+jj)*3+r, c] = vt[c,d]*hz[c,jj]*GDW^2*Wfr[c,r]
    k9_np = np.zeros((27, IN_C), np.float32)
    for d in range(3):
        for r in range(3):
            for jj in range(3):
                k9_np[(d * 3 + jj) * 3 + r, :] = (
                    vt[:, d] * hz[:, jj] * GDW * GDW * Wfr[:, r]
                )
    L = inputs["lr_weight0"][:, :, 0, 0].astype(np.float32) * np.float32(1.0 / np.sqrt(IN_C))
    Pp = inputs["point_w"][:, :, 0, 0].astype(np.float32) * np.float32(1.0 / np.sqrt(IN_C))
    plw3 = (Pp @ L @ Wfr).T                      # [3, 512]
    # whole linear path: z_pre = M27 @ s27, M27 = K27 P^T + PLW27
    m27_np = k9_np @ Pp.T                        # [27, 512]
    for r in range(3):
        m27_np[12 + r] += plw3[r]
    m27_np *= np.float32(ACT_SCALE)              # lrelu gain + FIR norms folded

    # ---- input replicas: [B, 108, 32*130] with all (chunk, dh, dw, rgb)
    # shifted windows pre-baked (partition p = 27*chunk + (dh*3+dw)*3 + rgb) ----
    spadflat = np.zeros((B, IMG_C, 130 * 130 + 2), np.float32)
    spad = np.zeros((B, IMG_C, 130, 130), np.float32)
    spad[:, :, 1:129, 1:129] = np.sin(img)
    spadflat[:, :, : 130 * 130] = spad.reshape(B, IMG_C, -1)
    st = spadflat.strides
    v = as_strided(
        spadflat,
        shape=(B, NCHUNK, 3, 3, IMG_C, HC, 130),
        strides=(st[0], 32 * 130 * st[2], 130 * st[2], st[2], st[1],
                 130 * st[2], st[2]),
    )
    s_np = np.zeros((B, 128, SW), np.float32)
    rep = v.reshape(B, NCHUNK, 27, SW)
    for j in range(NCHUNK):
        s_np[:, 32 * j : 32 * j + 27] = rep[:, j]
    s_np = s_np.astype(ml_dtypes.bfloat16)

    id_np = np.zeros((128, 256), np.float32)
    id_np[:, 0:128] = np.eye(128)
    id_np[:, 128:256] = 3.0 * np.eye(128)
    m27_rep = np.zeros((128, OUT_C), np.float32)
    for j in range(NCHUNK):
        m27_rep[32 * j : 32 * j + 27] = m27_np
    shared = dict(
        m27=m27_rep.astype(ml_dtypes.bfloat16),
        ident=id_np.astype(ml_dtypes.bfloat16),
    )
    in_maps = [dict(s=np.ascontiguousarray(s_np[c * B_LOC : (c + 1) * B_LOC]), **shared)
               for c in range(N_CORES)]

    if "nc" not in _CACHE:
        _CACHE["nc"] = _build_program()
    res = run_bass_kernel_spmd(_CACHE["nc"], in_maps, list(range(N_CORES)),
                               **_CACHE.get("run_kwargs", {}))
    _CACHE["last"] = res
    out = np.concatenate([res.results[c]["out"] for c in range(N_CORES)], axis=0)
    return out.astype(np.float32)
